# revision 14
# baseline (speedup 1.0000x reference)
"""Causal self-attention (H=1, B=4, T=4096, C=512) on 8 TRN2 NeuronCores.

Sharding: core = 2*b + h handles batch b with the parity-h half of the key
blocks (key block g of 128 rows goes to half h = g % 2).  Every core runs an
identical program (SPMD-uniform shapes):

  - projects the FULL query sequence of its batch (Q^T, [C, T] layout),
  - projects its 2048 packed key/value rows (K^T [C, 2048], V [2048, C]),
  - for each 512-query super-block i (8 of them) computes flash-style
    unnormalized attention against its first 2*(i+1) packed key blocks
    (exactly the causally-needed blocks of its parity; the 2 diagonal blocks
    get an additive -1e10 mask supplied as input data),
  - writes unnormalized O^T [C, T] and the per-query exp-sums [1, T].

Host side supplies q/k/v pre-transposed ([C, seq], with k/v parity-packed)
and combines the halves per batch: O = (O^T_0 + O^T_1)^T / (s_0 + s_1).

All matmuls run as float32r (tf32-class, 1 PE cycle/row at N=512).
"""

import numpy as np

B, T, C = 4, 4096, 512
HALF = T // 2            # packed key rows per core
NSB = T // 512           # 8 query super-blocks
SCALE = 1.0 / float(np.sqrt(C))
MASK_VAL = -1e10

_CACHE = {}
LAST_RESULTS = None


def _build_nc():
    import concourse.mybir as mybir
    from concourse import bacc
    from concourse.tile import TileContext

    F32 = mybir.dt.float32
    F32R = mybir.dt.float32r
    EXP = mybir.ActivationFunctionType.Exp
    IDENT = mybir.ActivationFunctionType.Identity

    nc = bacc.Bacc(trn_type="TRN2")

    qt_in = nc.dram_tensor("qT", [C, T], F32, kind="ExternalInput")
    kt_in = nc.dram_tensor("kxT", [C, HALF], F32, kind="ExternalInput")
    vt_in = nc.dram_tensor("vxT", [C, HALF], F32, kind="ExternalInput")
    w_in = {w: nc.dram_tensor(w, [C, C], F32, kind="ExternalInput")
            for w in ("Wq", "Wk", "Wv")}
    bq_in = nc.dram_tensor("bq2", [128, 4], F32, kind="ExternalInput")
    bk_in = nc.dram_tensor("bk2", [128, 4], F32, kind="ExternalInput")
    bv_in = nc.dram_tensor("bv1", [1, C], F32, kind="ExternalInput")
    mask_in = nc.dram_tensor("mask", [256, 512], F32, kind="ExternalInput")

    ot_out = nc.dram_tensor("OT", [C, T], F32, kind="ExternalOutput")
    ss_out = nc.dram_tensor("SS", [1, T], F32, kind="ExternalOutput")

    with TileContext(nc) as tc:
        with tc.sbuf_pool(name="persist", bufs=1) as pp:
            ones_f = pp.tile([128, 1], F32)
            nc.gpsimd.memset(ones_f, 1.0)
            ones_r = pp.tile([128, 1], F32R)
            nc.vector.tensor_copy(ones_r, ones_f)

            bq_t = pp.tile([128, 4], F32)
            nc.gpsimd.dma_start(bq_t, bq_in[:, :])
            bk_t = pp.tile([128, 4], F32)
            nc.gpsimd.dma_start(bk_t, bk_in[:, :])
            bv_row = pp.tile([1, C], F32)
            nc.gpsimd.dma_start(bv_row, bv_in[:, :])
            bv_bc = pp.tile([128, C], F32)
            nc.gpsimd.partition_broadcast(bv_bc, bv_row)

            mask_t = pp.tile([128, 1024], F32)

            # persistent projected tensors (f32r)
            QT = [pp.tile([128, T], F32R, name=f"QT{i}") for i in range(4)]
            KT = [pp.tile([128, HALF], F32R, name=f"KT{i}") for i in range(4)]
            V4 = [pp.tile([128, 4 * C], F32R, name=f"V4{i}") for i in range(4)]

            # ---------------- phase 1: load transposed + project ----------
            with tc.sbuf_pool(name="p1sb", bufs=1) as p1, \
                 tc.psum_pool(name="p1ps", bufs=1) as ps1:
                # weights straight to f32r via DMA bitcast (PE rounds on read)
                Wr = {}
                for w in ("Wq", "Wk", "Wv"):
                    tiles = []
                    for ci in range(4):
                        w_r = p1.tile([128, C], F32R, tag=f"w_{w}{ci}", bufs=1,
                                      name=f"{w}r{ci}")
                        eng = nc.sync if ci % 2 == 0 else nc.scalar
                        eng.dma_start(
                            w_r, w_in[w][ci * 128:(ci + 1) * 128, :].bitcast(F32R))
                        tiles.append(w_r)
                    Wr[w] = tiles

                def load_T_square(src, s, dma_eng=None):
                    """DMA the [512c, 512t] square at t-cols [s*512,(s+1)*512)
                    as 4 [128, 512] f32r tiles, split across both HWDGE
                    queues so each square lands in half the time."""
                    out = []
                    for cc in range(4):
                        xt = p1.tile([128, 512], F32R, tag="xt_sq", bufs=12,
                                     name=f"xt{s}{cc}")
                        eng = nc.sync if cc % 2 == 0 else nc.scalar
                        eng.dma_start(
                            xt, src[cc * 128:(cc + 1) * 128,
                                    s * 512:(s + 1) * 512].bitcast(F32R))
                        out.append(xt)
                    return out

                def do_q(s):
                    qt_sq = load_T_square(qt_in, s, nc.sync)
                    for co in range(4):
                        pj = ps1.tile([128, 512], F32, tag="proj", bufs=6,
                                      name=f"pjq{s}{co}")
                        for ci in range(4):
                            nc.tensor.matmul(
                                pj, Wr["Wq"][ci][:, co * 128:(co + 1) * 128],
                                qt_sq[ci], start=(ci == 0), stop=(ci == 3))
                        nc.vector.tensor_scalar_add(
                            QT[co][:, s * 512:(s + 1) * 512], pj,
                            bq_t[:, co:co + 1])

                def do_k(s):
                    kt_sq = load_T_square(kt_in, s, nc.scalar)
                    for co in range(4):
                        pj = ps1.tile([128, 512], F32, tag="proj", bufs=6,
                                      name=f"pjk{s}{co}")
                        for ci in range(4):
                            nc.tensor.matmul(
                                pj, Wr["Wk"][ci][:, co * 128:(co + 1) * 128],
                                kt_sq[ci], start=(ci == 0), stop=(ci == 3))
                        nc.vector.tensor_scalar_add(
                            KT[co][:, s * 512:(s + 1) * 512], pj,
                            bk_t[:, co:co + 1])

                def do_v(s):
                    vt_sq = load_T_square(vt_in, s, nc.scalar)
                    for t in range(4):
                        kb = s * 4 + t
                        pj = ps1.tile([128, 512], F32, tag="proj", bufs=6,
                                      name=f"pjv{s}{t}")
                        for ci in range(4):
                            nc.tensor.matmul(
                                pj, vt_sq[ci][:, t * 128:(t + 1) * 128],
                                Wr["Wv"][ci], start=(ci == 0), stop=(ci == 3))
                        nc.vector.tensor_add(
                            V4[kb // 4][:, (kb % 4) * 512:(kb % 4 + 1) * 512],
                            pj, bv_bc)

                # interleave so early attention super-blocks unblock ASAP
                do_q(0)
                do_k(0)
                do_v(0)
                nc.gpsimd.dma_start(mask_t[:, 0:512], mask_in[0:128, :])
                nc.gpsimd.dma_start(mask_t[:, 512:1024], mask_in[128:256, :])
                for s in range(1, NSB // 2):
                    do_k(s)
                    do_q(s)
                    do_v(s)
                for s in range(NSB // 2, NSB):
                    do_q(s)

            # ---------------- phase 2: attention ----------------
            with tc.sbuf_pool(name="p2sb", bufs=1) as p2, \
                 tc.psum_pool(name="p2ps", bufs=1) as ps2:
                for i in range(NSB):
                    nkb = 2 * (i + 1)
                    qs = slice(i * 512, (i + 1) * 512)
                    ot_ps = [ps2.tile([128, 512], F32, tag=f"ot{cc}", bufs=1,
                                      name=f"ot_ps{cc}")
                             for cc in range(4)]
                    sums_ps = ps2.tile([1, 512], F32, tag="sums", bufs=1)
                    sacc = p2.tile([128, 512], F32R, tag="sacc", bufs=2)
                    if nkb > 2:
                        kb_order = [0, nkb - 2, nkb - 1] + list(range(1, nkb - 2))
                    else:
                        kb_order = [0, 1]
                    for j, kb in enumerate(kb_order):
                        ks = slice(kb * 128, (kb + 1) * 128)
                        st = ps2.tile([128, 512], F32, tag="st", bufs=3)
                        for cc in range(4):
                            nc.tensor.matmul(st, KT[cc][:, ks], QT[cc][:, qs],
                                             start=(cc == 0), stop=(cc == 3))
                        if kb >= nkb - 2:
                            m = kb - (nkb - 2)
                            nc.vector.tensor_add(
                                st, st, mask_t[:, m * 512:(m + 1) * 512])
                        pt = p2.tile([128, 512], F32R, tag="pt", bufs=4)
                        nc.scalar.activation(pt, st, EXP, scale=SCALE)
                        first = j == 0
                        last = j == len(kb_order) - 1
                        for cc in range(4):
                            nc.tensor.matmul(
                                ot_ps[cc],
                                V4[kb // 4][:, (kb % 4) * 512 + cc * 128:
                                            (kb % 4) * 512 + (cc + 1) * 128],
                                pt, start=first, stop=last,
                                skip_group_check=True)
                        if first:
                            nc.vector.tensor_copy(sacc, pt)
                        else:
                            nc.vector.tensor_add(sacc, sacc, pt)
                    nc.tensor.matmul(sums_ps, ones_r, sacc,
                                     start=True, stop=True,
                                     skip_group_check=True)
                    for cc in range(4):
                        ot_sb = p2.tile([128, 512], F32, tag="otsb", bufs=4)
                        if cc % 2 == 0:
                            nc.scalar.copy(ot_sb, ot_ps[cc])
                        else:
                            nc.vector.tensor_copy(ot_sb, ot_ps[cc])
                        (nc.sync if cc % 2 == 0 else nc.scalar).dma_start(
                            ot_out[cc * 128:(cc + 1) * 128, qs], ot_sb)
                    ss_sb = p2.tile([1, 512], F32, tag="sssb", bufs=2)
                    nc.vector.tensor_copy(ss_sb, sums_ps)
                    nc.scalar.dma_start(ss_out[0:1, qs], ss_sb)

    nc.compile()
    return nc


def _prep_inputs(q, k, v, Wq, bq, Wk, bk, Wv, bv):
    """Build the 8 per-core input maps (inputs pre-transposed on host)."""
    in_maps = []
    bq2 = np.ascontiguousarray(bq.reshape(4, 128).T)
    bk2 = np.ascontiguousarray(bk.reshape(4, 128).T)
    for core in range(8):
        b, h = core // 2, core % 2
        idx = np.arange(16) * 2 + h            # parity key blocks
        rows = (idx[:, None] * 128 + np.arange(128)[None, :]).reshape(-1)
        mask = np.zeros((256, 512), np.float32)
        kk = np.arange(256)
        kglob = (2 * (kk // 128) + h) * 128 + (kk % 128)
        mask[np.arange(512)[None, :] < kglob[:, None]] = MASK_VAL
        in_maps.append({
            "qT": np.ascontiguousarray(q[b].T),
            "kxT": np.ascontiguousarray(k[b][rows].T),
            "vxT": np.ascontiguousarray(v[b][rows].T),
            "Wq": Wq, "Wk": Wk, "Wv": Wv,
            "bq2": bq2, "bk2": bk2,
            "bv1": bv.reshape(1, C),
            "mask": mask,
        })
    return in_maps


def kernel(**inputs):
    global LAST_RESULTS
    from concourse.bass_utils import run_bass_kernel_spmd

    arrs = {n: np.asarray(inputs[n], dtype=np.float32)
            for n in ("q", "k", "v", "Wq", "bq", "Wk", "bk", "Wv", "bv")}
    if "nc" not in _CACHE:
        _CACHE["nc"] = _build_nc()
    nc = _CACHE["nc"]

    in_maps = _prep_inputs(**arrs)
    res = run_bass_kernel_spmd(nc, in_maps, core_ids=list(range(8)))
    LAST_RESULTS = res

    out = np.empty((B, T, C), np.float32)
    for b in range(B):
        ot = (res.results[2 * b]["OT"].astype(np.float64)
              + res.results[2 * b + 1]["OT"].astype(np.float64))
        s = (res.results[2 * b]["SS"][0].astype(np.float64)
             + res.results[2 * b + 1]["SS"][0].astype(np.float64))
        out[b] = (ot.T / s[:, None]).astype(np.float32)
    return out


# revision 15
# speedup vs baseline: 1.0126x; 1.0126x over previous
"""Causal self-attention (H=1, B=4, T=4096, C=512) on 8 TRN2 NeuronCores.

Sharding: core = 2*b + h handles batch b with the parity-h half of the key
blocks (key block g of 128 rows goes to half h = g % 2).  Every core runs an
identical program (SPMD-uniform shapes):

  - projects the FULL query sequence of its batch (Q^T, [C, T] layout),
  - projects its 2048 packed key/value rows (K^T [C, 2048], V [2048, C]),
  - for each 512-query super-block i (8 of them) computes flash-style
    unnormalized attention against its first 2*(i+1) packed key blocks
    (exactly the causally-needed blocks of its parity; the 2 diagonal blocks
    get an additive -1e10 mask supplied as input data),
  - writes unnormalized O^T [C, T] and the per-query exp-sums [1, T].

Host side supplies q/k/v pre-transposed ([C, seq], with k/v parity-packed)
and combines the halves per batch: O = (O^T_0 + O^T_1)^T / (s_0 + s_1).

All matmuls run as float32r (tf32-class, 1 PE cycle/row at N=512).
"""

import numpy as np

B, T, C = 4, 4096, 512
HALF = T // 2            # packed key rows per core
NSB = T // 512           # 8 query super-blocks
SCALE = 1.0 / float(np.sqrt(C))
MASK_VAL = -1e10

_CACHE = {}
LAST_RESULTS = None


def _build_nc():
    import concourse.mybir as mybir
    from concourse import bacc
    from concourse.tile import TileContext

    F32 = mybir.dt.float32
    F32R = mybir.dt.float32r
    EXP = mybir.ActivationFunctionType.Exp
    IDENT = mybir.ActivationFunctionType.Identity

    nc = bacc.Bacc(trn_type="TRN2")

    qt_in = nc.dram_tensor("qT", [C, T], F32, kind="ExternalInput")
    kt_in = nc.dram_tensor("kxT", [C, HALF], F32, kind="ExternalInput")
    vt_in = nc.dram_tensor("vxT", [C, HALF], F32, kind="ExternalInput")
    w_in = {w: nc.dram_tensor(w, [C, C], F32, kind="ExternalInput")
            for w in ("Wq", "Wk", "Wv")}
    bq_in = nc.dram_tensor("bq2", [128, 4], F32, kind="ExternalInput")
    bk_in = nc.dram_tensor("bk2", [128, 4], F32, kind="ExternalInput")
    bv_in = nc.dram_tensor("bvb", [128, C], F32, kind="ExternalInput")
    mask_in = nc.dram_tensor("mask", [256, 512], F32, kind="ExternalInput")

    ot_out = nc.dram_tensor("OT", [C, T], F32, kind="ExternalOutput")
    ss_out = nc.dram_tensor("SS", [1, T], F32, kind="ExternalOutput")

    with TileContext(nc) as tc:
        with tc.sbuf_pool(name="persist", bufs=1) as pp:
            ones_f = pp.tile([128, 1], F32)
            nc.gpsimd.memset(ones_f, 1.0)
            ones_r = pp.tile([128, 1], F32R)
            nc.vector.tensor_copy(ones_r, ones_f)

            bq_t = pp.tile([128, 4], F32)
            nc.gpsimd.dma_start(bq_t, bq_in[:, :])
            bk_t = pp.tile([128, 4], F32)
            nc.gpsimd.dma_start(bk_t, bk_in[:, :])
            bv_bc = pp.tile([128, C], F32)
            nc.gpsimd.dma_start(bv_bc, bv_in[:, :])

            mask_t = pp.tile([128, 1024], F32)

            # persistent projected tensors (f32r)
            QT = [pp.tile([128, T], F32R, name=f"QT{i}") for i in range(4)]
            KT = [pp.tile([128, HALF], F32R, name=f"KT{i}") for i in range(4)]
            V4 = [pp.tile([128, 4 * C], F32R, name=f"V4{i}") for i in range(4)]

            # ---------------- phase 1: load transposed + project ----------
            with tc.sbuf_pool(name="p1sb", bufs=1) as p1, \
                 tc.psum_pool(name="p1ps", bufs=1) as ps1:
                # weights straight to f32r via DMA bitcast (PE rounds on read)
                Wr = {}

                def load_w(w):
                    tiles = []
                    for ci in range(4):
                        w_r = p1.tile([128, C], F32R, tag=f"w_{w}{ci}", bufs=1,
                                      name=f"{w}r{ci}")
                        eng = nc.sync if ci % 2 == 0 else nc.scalar
                        eng.dma_start(
                            w_r, w_in[w][ci * 128:(ci + 1) * 128, :].bitcast(F32R))
                        tiles.append(w_r)
                    Wr[w] = tiles

                def load_T_square(src, s, dma_eng=None):
                    """DMA the [512c, 512t] square at t-cols [s*512,(s+1)*512)
                    as 4 [128, 512] f32r tiles, split across both HWDGE
                    queues so each square lands in half the time."""
                    out = []
                    for cc in range(4):
                        xt = p1.tile([128, 512], F32R, tag="xt_sq", bufs=12,
                                     name=f"xt{s}{cc}")
                        eng = nc.sync if cc % 2 == 0 else nc.scalar
                        eng.dma_start(
                            xt, src[cc * 128:(cc + 1) * 128,
                                    s * 512:(s + 1) * 512].bitcast(F32R))
                        out.append(xt)
                    return out

                def do_q(s):
                    qt_sq = load_T_square(qt_in, s, nc.sync)
                    for co in range(4):
                        pj = ps1.tile([128, 512], F32, tag="proj", bufs=6,
                                      name=f"pjq{s}{co}")
                        for ci in range(4):
                            nc.tensor.matmul(
                                pj, Wr["Wq"][ci][:, co * 128:(co + 1) * 128],
                                qt_sq[ci], start=(ci == 0), stop=(ci == 3))
                        nc.vector.tensor_scalar_add(
                            QT[co][:, s * 512:(s + 1) * 512], pj,
                            bq_t[:, co:co + 1])

                def do_k(s):
                    kt_sq = load_T_square(kt_in, s, nc.scalar)
                    for co in range(4):
                        pj = ps1.tile([128, 512], F32, tag="proj", bufs=6,
                                      name=f"pjk{s}{co}")
                        for ci in range(4):
                            nc.tensor.matmul(
                                pj, Wr["Wk"][ci][:, co * 128:(co + 1) * 128],
                                kt_sq[ci], start=(ci == 0), stop=(ci == 3))
                        nc.vector.tensor_scalar_add(
                            KT[co][:, s * 512:(s + 1) * 512], pj,
                            bk_t[:, co:co + 1])

                def do_v(s):
                    vt_sq = load_T_square(vt_in, s, nc.scalar)
                    for t in range(4):
                        kb = s * 4 + t
                        pj = ps1.tile([128, 512], F32, tag="proj", bufs=6,
                                      name=f"pjv{s}{t}")
                        for ci in range(4):
                            nc.tensor.matmul(
                                pj, vt_sq[ci][:, t * 128:(t + 1) * 128],
                                Wr["Wv"][ci], start=(ci == 0), stop=(ci == 3))
                        nc.vector.tensor_add(
                            V4[kb // 4][:, (kb % 4) * 512:(kb % 4 + 1) * 512],
                            pj, bv_bc)

                # interleave so early attention super-blocks unblock ASAP
                load_w("Wq")
                do_q(0)
                load_w("Wk")
                do_k(0)
                load_w("Wv")
                do_v(0)
                nc.gpsimd.dma_start(mask_t[:, 0:512], mask_in[0:128, :])
                nc.gpsimd.dma_start(mask_t[:, 512:1024], mask_in[128:256, :])
                for s in range(1, NSB // 2):
                    do_k(s)
                    do_q(s)
                    do_v(s)
                for s in range(NSB // 2, NSB):
                    do_q(s)

            # ---------------- phase 2: attention ----------------
            with tc.sbuf_pool(name="p2sb", bufs=1) as p2, \
                 tc.psum_pool(name="p2ps", bufs=1) as ps2:
                for i in range(NSB):
                    nkb = 2 * (i + 1)
                    qs = slice(i * 512, (i + 1) * 512)
                    ot_ps = [ps2.tile([128, 512], F32, tag=f"ot{cc}", bufs=1,
                                      name=f"ot_ps{cc}")
                             for cc in range(4)]
                    sums_ps = ps2.tile([1, 512], F32, tag="sums", bufs=1)
                    sacc = p2.tile([128, 512], F32R, tag="sacc", bufs=2)
                    if nkb > 2:
                        kb_order = [0, nkb - 2, nkb - 1] + list(range(1, nkb - 2))
                    else:
                        kb_order = [0, 1]
                    for j, kb in enumerate(kb_order):
                        ks = slice(kb * 128, (kb + 1) * 128)
                        st = ps2.tile([128, 512], F32, tag="st", bufs=3)
                        for cc in range(4):
                            nc.tensor.matmul(st, KT[cc][:, ks], QT[cc][:, qs],
                                             start=(cc == 0), stop=(cc == 3))
                        if kb >= nkb - 2:
                            m = kb - (nkb - 2)
                            nc.vector.tensor_add(
                                st, st, mask_t[:, m * 512:(m + 1) * 512])
                        pt = p2.tile([128, 512], F32R, tag="pt", bufs=4)
                        nc.scalar.activation(pt, st, EXP, scale=SCALE)
                        first = j == 0
                        last = j == len(kb_order) - 1
                        for cc in range(4):
                            nc.tensor.matmul(
                                ot_ps[cc],
                                V4[kb // 4][:, (kb % 4) * 512 + cc * 128:
                                            (kb % 4) * 512 + (cc + 1) * 128],
                                pt, start=first, stop=last,
                                skip_group_check=True)
                        if i == NSB - 1 and last:
                            # keep the final pt out of sacc; sum it directly
                            # so the tail isn't DVE-add -> MM serialized
                            nc.tensor.matmul(sums_ps, ones_r, pt,
                                             start=False, stop=True,
                                             skip_group_check=True)
                        elif first:
                            nc.vector.tensor_copy(sacc, pt)
                        else:
                            nc.vector.tensor_add(sacc, sacc, pt)
                            if i == NSB - 1 and j == len(kb_order) - 2:
                                nc.tensor.matmul(sums_ps, ones_r, sacc,
                                                 start=True, stop=False,
                                                 skip_group_check=True)
                    if i != NSB - 1:
                        nc.tensor.matmul(sums_ps, ones_r, sacc,
                                         start=True, stop=True,
                                         skip_group_check=True)
                    for cc in range(4):
                        ot_sb = p2.tile([128, 512], F32, tag="otsb", bufs=4)
                        if cc % 2 == 0:
                            nc.scalar.copy(ot_sb, ot_ps[cc])
                        else:
                            nc.vector.tensor_copy(ot_sb, ot_ps[cc])
                        (nc.sync if cc % 2 == 0 else nc.scalar).dma_start(
                            ot_out[cc * 128:(cc + 1) * 128, qs], ot_sb)
                    ss_sb = p2.tile([1, 512], F32, tag="sssb", bufs=2)
                    nc.vector.tensor_copy(ss_sb, sums_ps)
                    nc.scalar.dma_start(ss_out[0:1, qs], ss_sb)

    nc.compile()
    return nc


def _prep_inputs(q, k, v, Wq, bq, Wk, bk, Wv, bv):
    """Build the 8 per-core input maps (inputs pre-transposed on host)."""
    in_maps = []
    bq2 = np.ascontiguousarray(bq.reshape(4, 128).T)
    bk2 = np.ascontiguousarray(bk.reshape(4, 128).T)
    for core in range(8):
        b, h = core // 2, core % 2
        idx = np.arange(16) * 2 + h            # parity key blocks
        rows = (idx[:, None] * 128 + np.arange(128)[None, :]).reshape(-1)
        mask = np.zeros((256, 512), np.float32)
        kk = np.arange(256)
        kglob = (2 * (kk // 128) + h) * 128 + (kk % 128)
        mask[np.arange(512)[None, :] < kglob[:, None]] = MASK_VAL
        in_maps.append({
            "qT": np.ascontiguousarray(q[b].T),
            "kxT": np.ascontiguousarray(k[b][rows].T),
            "vxT": np.ascontiguousarray(v[b][rows].T),
            "Wq": Wq, "Wk": Wk, "Wv": Wv,
            "bq2": bq2, "bk2": bk2,
            "bvb": np.ascontiguousarray(np.broadcast_to(bv, (128, C))),
            "mask": mask,
        })
    return in_maps


def kernel(**inputs):
    global LAST_RESULTS
    from concourse.bass_utils import run_bass_kernel_spmd

    arrs = {n: np.asarray(inputs[n], dtype=np.float32)
            for n in ("q", "k", "v", "Wq", "bq", "Wk", "bk", "Wv", "bv")}
    if "nc" not in _CACHE:
        _CACHE["nc"] = _build_nc()
    nc = _CACHE["nc"]

    in_maps = _prep_inputs(**arrs)
    res = run_bass_kernel_spmd(nc, in_maps, core_ids=list(range(8)))
    LAST_RESULTS = res

    out = np.empty((B, T, C), np.float32)
    for b in range(B):
        ot = (res.results[2 * b]["OT"].astype(np.float64)
              + res.results[2 * b + 1]["OT"].astype(np.float64))
        s = (res.results[2 * b]["SS"][0].astype(np.float64)
             + res.results[2 * b + 1]["SS"][0].astype(np.float64))
        out[b] = (ot.T / s[:, None]).astype(np.float32)
    return out
